# revision 2
# baseline (speedup 1.0000x reference)
"""Linear (kernel-feature) attention for Trainium2, sharded over 8 NeuronCores.

Problem: B=4, H=16, S=4096, D=64 fp32.
    phi(x) = elu(x) + 1  (= min(exp(x),1) + relu(x))
    kv   = phi_k_masked^T @ V          [d, v]
    k1   = phi_k^T @ mask              [d]
    out  = (phi_q @ kv) / (phi_q @ k1 + eps)

Sharding: 64 (b,h) slices -> 8 per core, processed as 4 pairs.
All device data is bf16 (tolerance is 2e-2; measured end-to-end error of the
bf16 pipeline is ~3.3e-3), halving the HBM roofline to ~16 MB/core and
doubling PE stream rate + enabling FWL weight loads.

Host-side layout (part of sharding, costs no HW time):
  - qT : [PAIRS, 128, S]       pair tile, partitions 0:64 = slice a's q^T
         [d, n], 64:128 = slice b's. phi_q's 1/8 scale is applied by ACT.
  - kp : [PAIRS, 128, NT*128]  per 128-row n-tile t, cols t*128+{0:64,64:128}
         = slice {a,b}'s K tile [n-part, d]. One 128-col FWL weight load
         per tile serves both slices' M1 matmuls.
  - vx : [PAIRS, 128, NT*130]  per tile, cols t*130+{0:65,65:130} = slice
         {a,b}'s [V | mask] (mask column fuses k1 into the kv matmul; the
         attention mask is folded into V and this column on host).
  - outc:[PAIRS, 128, 2*FREE]  natural [n-part, t, d] layout per slice,
         slice a at cols 0:FREE, b at FREE:2*FREE. bf16; host upcasts.

Device per pair:
  phi_k: ACT exp + DVE relu + DVE min/add combine (in place)
  M1: 32 accumulating matmuls lhsT=[128,128] (FWL), rhs=[128,130] ->
      kv_ext psum [128,130]; valid quadrants [0:64,0:65]=a, [64:,65:]=b.
  phi_q on transposed layout (exp scale=0.125 on ACT, (q*0.125) max 0 on DVE)
  M2: per slice, per 7-tile psum group: one matmul per n-tile with fused
      [kv|k1] rhs (N=65) -> psum [128, 7*65]; DVE adds eps + reciprocal on
      the strided nrm columns, then one broadcast multiply evacuates
      psum -> bf16 out tile.
"""

import sys

sys.path.insert(0, "/opt/trn_rl_repo")

import numpy as np
import ml_dtypes

B, H, S, D = 4, 16, 4096, 64
N_CORES = 8
SL = (B * H) // N_CORES  # slices per core = 8
PAIRS = SL // 2  # 4
NT = S // 128  # 32 n-tiles per slice
FREE = NT * D  # 2048 free cols per slice of output
EPS = 1e-6
BF16 = ml_dtypes.bfloat16

_programs: dict = {}


def _build_program():
    from contextlib import ExitStack

    import concourse.bacc as bacc
    import concourse.tile as tile
    from concourse import mybir

    f32 = mybir.dt.float32
    bf16 = mybir.dt.bfloat16
    Alu = mybir.AluOpType
    Act = mybir.ActivationFunctionType

    nc = bacc.Bacc("TRN2", target_bir_lowering=False, debug=False)
    qT = nc.dram_tensor("qT", [PAIRS, 128, S], bf16, kind="ExternalInput").ap()
    kp = nc.dram_tensor("kp", [PAIRS, 128, NT * 128], bf16, kind="ExternalInput").ap()
    vx = nc.dram_tensor("vx", [PAIRS, 128, NT * 130], bf16, kind="ExternalInput").ap()
    outc = nc.dram_tensor(
        "outc", [PAIRS, 128, 2 * FREE], bf16, kind="ExternalOutput"
    ).ap()

    # M2 tile groups: psum bank holds 7 tiles of 65 cols (455 <= 512)
    GROUPS = [(0, 7), (7, 7), (14, 7), (21, 7), (28, 4)]

    with tile.TileContext(nc) as tc, ExitStack() as ctx:
        kpool = ctx.enter_context(tc.tile_pool(name="kpool", bufs=2))
        vpool = ctx.enter_context(tc.tile_pool(name="vpool", bufs=2))
        qpool = ctx.enter_context(tc.tile_pool(name="qpool", bufs=2))
        tmp = ctx.enter_context(tc.tile_pool(name="tmp", bufs=2))
        kvp = ctx.enter_context(tc.tile_pool(name="kvp", bufs=2))
        nrmp = ctx.enter_context(tc.tile_pool(name="nrmp", bufs=4))
        outp = ctx.enter_context(tc.tile_pool(name="outp", bufs=2))
        ps_kv = ctx.enter_context(tc.tile_pool(name="ps_kv", bufs=2, space="PSUM"))
        ps_out = ctx.enter_context(tc.tile_pool(name="ps_out", bufs=4, space="PSUM"))

        for pair in range(PAIRS):
            kt = kpool.tile([128, NT * 128], bf16)
            nc.sync.dma_start(out=kt, in_=kp[pair])
            vt = vpool.tile([128, NT * 130], bf16)
            nc.sync.dma_start(out=vt, in_=vx[pair])
            qt = qpool.tile([128, S], bf16)
            nc.sync.dma_start(out=qt, in_=qT[pair])

            # ---- phi_k = min(exp(k),1) + relu(k), in place in kt
            e = tmp.tile([128, S], bf16, tag="e")
            nc.scalar.activation(e, kt, Act.Exp)
            r = tmp.tile([128, S], bf16, tag="r")
            nc.vector.tensor_scalar_max(r, kt, 0.0)
            nc.vector.scalar_tensor_tensor(kt, e, 1.0, r, Alu.min, Alu.add)

            # ---- M1: kv_ext for both slices in one accumulation chain
            kv_ps = ps_kv.tile([128, 130], f32)
            for t in range(NT):
                nc.tensor.matmul(
                    kv_ps,
                    kt[:, t * 128 : (t + 1) * 128],
                    vt[:, t * 130 : (t + 1) * 130],
                    start=(t == 0),
                    stop=(t == NT - 1),
                )
            kv_sb = kvp.tile([128, 130], bf16)
            nc.vector.tensor_copy(kv_sb, kv_ps)

            # ---- phi_q = min(exp(q/8),1) + relu(q)/8, in place in qt
            e2 = tmp.tile([128, S], bf16, tag="e")
            nc.scalar.activation(e2, qt, Act.Exp, scale=0.125)
            r2 = tmp.tile([128, S], bf16, tag="r")
            nc.vector.tensor_scalar(r2, qt, 0.125, 0.0, Alu.mult, Alu.max)
            nc.vector.scalar_tensor_tensor(qt, e2, 1.0, r2, Alu.min, Alu.add)

            # ---- M2 + divide + store
            out_sb = outp.tile([128, 2 * FREE], bf16)
            for rr in range(2):
                rhs = (
                    kv_sb[0:64, 0:65] if rr == 0 else kv_sb[64:128, 65:130]
                )  # [kv | k1] for this slice
                for g0, gs in GROUPS:
                    po = ps_out.tile([128, 455], f32, tag="po")
                    for i in range(gs):
                        t = g0 + i
                        nc.tensor.matmul(
                            po[:, i * 65 : (i + 1) * 65],
                            qt[64 * rr : 64 * rr + 64, t * 128 : (t + 1) * 128],
                            rhs,
                            start=(i == 0),
                            stop=(i == gs - 1),
                            tile_position=(64 * rr, 0),
                            skip_group_check=True,
                        )
                    po3 = po[:, 0 : gs * 65].rearrange("p (t c) -> p t c", c=65)
                    ns = nrmp.tile([128, 7], f32)
                    ns3 = ns[:, 0:gs].rearrange("p (t o) -> p t o", o=1)
                    nc.vector.tensor_scalar_add(ns3, po3[:, :, 64:65], EPS)
                    nc.vector.reciprocal(ns[:, 0:gs], ns[:, 0:gs])
                    dst = out_sb[
                        :, rr * FREE + g0 * 64 : rr * FREE + (g0 + gs) * 64
                    ].rearrange("p (t c) -> p t c", c=64)
                    nc.vector.tensor_tensor(
                        dst,
                        po3[:, :, 0:64],
                        ns[:, 0:gs].broadcast_to([128, gs, 64]),
                        Alu.mult,
                    )
            nc.sync.dma_start(out=outc[pair], in_=out_sb)

    nc.compile()
    return nc


def _get_program():
    if "p" not in _programs:
        _programs["p"] = _build_program()
    return _programs["p"]


def _pack_inputs(query, key, value, attention_mask):
    """Shard + lay out inputs for the 8 cores (all bf16)."""
    q4 = np.asarray(query, dtype=np.float32).reshape(B * H, S, D)
    k4 = np.asarray(key, dtype=np.float32).reshape(B * H, S, D)
    v4 = np.asarray(value, dtype=np.float32).reshape(B * H, S, D)
    am = np.asarray(attention_mask, dtype=np.float32)

    # qT: [g, d, n] -> [cores, PAIRS, 128, S]
    qT = (
        np.ascontiguousarray(q4.transpose(0, 2, 1))
        .reshape(N_CORES, PAIRS, 2 * D, S)
        .astype(BF16)
    )
    # kp: [g, n, d] -> [cores, pair, p, t*128 + s*64 + d]
    k6 = k4.reshape(N_CORES, PAIRS, 2, NT, 128, D)
    kpl = np.ascontiguousarray(k6.transpose(0, 1, 4, 3, 2, 5)).reshape(
        N_CORES, PAIRS, 128, NT * 128
    ).astype(BF16)
    # vx: [V*mask | mask] per slice -> [cores, pair, p, t*130 + s*65 + c]
    # mask fold: kv = phi_k^T (mask*V), k1 = phi_k^T mask
    mrow = np.repeat(am, H, axis=0).reshape(B * H, S, 1)  # [g, n, 1]
    if np.all(am == 1.0):
        vext = np.concatenate([v4, mrow], axis=-1)
    else:
        vext = np.concatenate([v4 * mrow, mrow], axis=-1)
    v6 = vext.reshape(N_CORES, PAIRS, 2, NT, 128, D + 1)
    vxl = np.ascontiguousarray(v6.transpose(0, 1, 4, 3, 2, 5)).reshape(
        N_CORES, PAIRS, 128, NT * 130
    ).astype(BF16)

    return [
        {"qT": qT[c], "kp": kpl[c], "vx": vxl[c]} for c in range(N_CORES)
    ]


def _unpack_output(results):
    outs = np.stack([r["outc"] for r in results])  # [cores, PAIRS, 128, 2*FREE]
    outs = outs.astype(np.float32).reshape(N_CORES, PAIRS, 128, 2, NT, D)
    outs = outs.transpose(0, 1, 3, 4, 2, 5)  # [cores, pair, s, t, p, d]
    return np.ascontiguousarray(outs).reshape(B, H, S, D)


def kernel(query, key, value, attention_mask):
    from concourse.bass_utils import run_bass_kernel_spmd

    in_maps = _pack_inputs(query, key, value, attention_mask)
    nc = _get_program()
    res = run_bass_kernel_spmd(nc, in_maps, core_ids=list(range(N_CORES)))
    return _unpack_output(res.results)
